# revision 2
# baseline (speedup 1.0000x reference)
"""Trainium2 Bass kernel v3 for nn_DenseFlashAttention_16123307229343
(GNN segment-softmax message passing). kernel(**inputs) -> np.ndarray.

v3 redesign vs v2 (one-hot scatter baseline):
- ELL layout: receivers globally degree-sorted into chunks of 128; tile k
  of a chunk holds the k-th edge of each of the chunk's 128 receivers, so
  receiver == partition slot. One-hot scatter matrices are gone entirely
  (16.4 MB DMA/core + 2 of 3 matmuls/tile eliminated); segment sums become
  free-dim reductions / identity-matmul PSUM accumulation.
- Normalize-before-scatter: alpha = w/d per edge via per-partition
  broadcast (d is per-slot), which merges tangential+radial per head into
  beta (4 blocks instead of 8) and shrinks the scatter payload 2x.
- Raw-x payload: the per-edge 264-wide projection matmul is replaced by an
  8-col score matmul; the per-head projection (folded with w_out) is
  applied per chunk (128 receivers) after the reduction, via 2 stacked
  128-contraction matmuls on transposed U.
- Chunk K-profile is shared across all 8 cores (round-robin deal by degree
  rank, padded to the round max), so one SPMD program serves all cores
  with ~0.5% padding.
"""
import sys, math
sys.path.insert(0, '/opt/trn_rl_repo')

import numpy as np
import ml_dtypes

import concourse.mybir as mybir
import concourse.bass as bass
from concourse.tile import TileContext
from concourse.vector_clock import ScopedClock

bf16 = ml_dtypes.bfloat16
FP32 = mybir.dt.float32
BF16 = mybir.dt.bfloat16
ALU = mybir.AluOpType
ACTF = mybir.ActivationFunctionType
AXIS = mybir.AxisListType

MAXW = 1


def _patched_drain_and_barrier(self, tick_clock, wait_clock):
    nc = self.nc
    drain_inst = nc.sync.drain()
    wait_clock.add_sem_waits(drain_inst.ins, ScopedClock({None: tick_clock.global_clock}))
    si = drain_inst.ins.sync_info
    waits = list(si.on_wait) if si is not None else []
    if len(waits) > MAXW:
        si.on_wait = waits[:MAXW]
        rest = waits[MAXW:]
        for i in range(0, len(rest), MAXW):
            d2 = nc.sync.drain()
            d2.ins.sync_info = mybir.SyncInfo(on_wait=rest[i:i+MAXW], on_update=[])
    nc.all_engine_barrier()
    popped = nc._tile_sem_poison_stack.pop()
    assert popped is self._sem_poison
    nc.clear_and_free_semaphores(list(self.sems.allocated().values()))
    nc.all_engine_barrier()


def install():
    TileContext._drain_and_barrier = _patched_drain_and_barrier


_ctr = [0]


def split_sync_waits(nc, maxw=1):
    """The walrus build in this container rejects instructions carrying more
    than one sync wait. Hoist extra waits onto carriers inserted immediately
    before the instruction on the same engine."""
    for f in nc.m.functions:
        for blk in f.blocks:
            lst = blk.instructions
            i = 0
            while i < len(lst):
                ins = lst[i]
                si = ins.sync_info
                if si is None:
                    i += 1
                    continue
                waits = list(si.on_wait)
                if len(waits) <= maxw:
                    i += 1
                    continue
                si.on_wait = waits[-maxw:]
                rest = waits[:-maxw]
                carriers = []
                for j in range(0, len(rest), maxw):
                    _ctr[0] += 1
                    nop = mybir.InstEventSemaphore(name=f"waitnop_{_ctr[0]}", ins=[], outs=[])
                    nop.engine = ins.engine
                    nop.sync_info = mybir.SyncInfo(on_wait=rest[j:j + maxw],
                                                   on_update=[])
                    nc.register_instruction(nop, overwrite=True)
                    carriers.append(nop)
                for k, nop in enumerate(carriers):
                    lst.insert(i + k, nop)
                i += len(carriers) + 1


NC_CORES = 8


def host_prep(x, edge_index, edge_len, F=64):
    """ELL layout with a core-shared K profile. Returns per-core input
    tensors plus the metadata needed to assemble the output."""
    N = x.shape[0]
    E = edge_index.shape[1]
    snd = edge_index[0].astype(np.int64)
    rcv = edge_index[1].astype(np.int64)
    deg = np.bincount(rcv, minlength=N)
    eorder = np.argsort(rcv, kind='stable')
    starts = np.zeros(N + 1, np.int64)
    np.cumsum(deg, out=starts[1:])

    rorder = np.argsort(-deg, kind='stable')
    nch_total = (N + 127) // 128
    # pad chunk count so the per-core round count is even (pair processing)
    nch_pad = ((nch_total + 2 * NC_CORES - 1) // (2 * NC_CORES)) * 2 * NC_CORES
    rounds = nch_pad // NC_CORES
    recv_of_chunk = np.full((nch_pad, 128), -1, np.int64)
    recv_of_chunk.reshape(-1)[:N] = rorder
    # chunk max degree; profile = per-round max so all cores share it
    degc = np.where(recv_of_chunk >= 0, deg[np.clip(recv_of_chunk, 0, N - 1)], 0)
    Kc = np.maximum(degc.max(axis=1), 1)
    Khat = np.array([int(Kc[r * NC_CORES:(r + 1) * NC_CORES].max())
                     for r in range(rounds)], np.int64)
    # ascending-K round order: pipeline warms up and drains on small chunks
    rord = np.argsort(Khat, kind='stable')
    Khat = Khat[rord]
    # pair adjacent rounds; both chunks of a pair padded to the pair max
    Kpair = np.maximum(Khat[0::2], Khat[1::2])
    Khat = np.repeat(Kpair, 2)
    T_tot = int(Khat.sum())

    xb = np.zeros((N + 1, F), dtype=bf16)
    xb[:N] = x.astype(bf16)
    lenb = edge_len.astype(np.float32)

    offs = np.zeros(rounds + 1, np.int64)
    np.cumsum(Khat, out=offs[1:])

    per_core = []
    for core in range(NC_CORES):
        xeT = np.zeros((F + 2, T_tot * 128), dtype=bf16)
        xE = np.zeros((128, T_tot * F), dtype=bf16)
        xr = np.zeros((rounds * 128, F), np.float32)
        recvs = np.empty((rounds, 128), np.int64)
        for r in range(rounds):
            c = core + int(rord[r]) * NC_CORES
            K = int(Khat[r])
            n = recv_of_chunk[c]                       # [128]
            recvs[r] = n
            nc_ = np.clip(n, 0, N - 1)
            d = np.where(n >= 0, deg[nc_], 0)          # [128]
            ks = np.arange(K)
            valid = ks[None, :] < d[:, None]           # [128, K]
            eidx = starts[nc_][:, None] + np.minimum(ks[None, :], np.maximum(d[:, None] - 1, 0))
            e = eorder[np.clip(eidx, 0, E - 1)]
            sg = np.where(valid, snd[e], N)            # pad -> row N (zeros)
            blk = xb[sg]                               # [128, K, F] bf16
            lg = np.where(valid, lenb[e], 0.0).astype(bf16)
            mk = (~valid).astype(bf16)
            o = offs[r]
            xE[:, o * F:(o + K) * F] = blk.reshape(128, K * F)
            xeT[0:F, o * 128:(o + K) * 128] = blk.transpose(2, 1, 0).reshape(F, K * 128)
            xeT[F, o * 128:(o + K) * 128] = lg.T.reshape(K * 128)
            xeT[F + 1, o * 128:(o + K) * 128] = mk.T.reshape(K * 128)
            has = n >= 0
            xr[r * 128:(r + 1) * 128][has] = x[n[has]]
        xrT = np.ascontiguousarray(xr.T).astype(bf16)
        per_core.append(dict(xeT=xeT, xE=xE, x_rcvT=xrT, recvs=recvs))
    return dict(Khat=tuple(int(k) for k in Khat), rounds=rounds, T_tot=T_tot,
                offs=offs, per_core=per_core, N=N, F=F)


def build_program(Khat, rounds, T_tot, F=64, H=4):
    nc = bass.Bass("TRN2", target_bir_lowering=False, debug=False,
                   num_devices=NC_CORES)
    FW = F + 2                       # 66 contraction rows (x, len, mask)
    J = 2 * H                        # 8 score columns
    S = rounds * 128                 # receiver slots per core

    xeT = nc.dram_tensor("xeT", [FW, T_tot * 128], BF16, kind="ExternalInput").ap()
    xE = nc.dram_tensor("xE", [128, T_tot * F], BF16, kind="ExternalInput").ap()
    wv10 = nc.dram_tensor("wv10", [FW, J], BF16, kind="ExternalInput").ap()
    mcat = nc.dram_tensor("mcat", [128, 2 * F], BF16, kind="ExternalInput").ap()
    wm2 = nc.dram_tensor("wm2", [F, F], BF16, kind="ExternalInput").ap()
    ident = nc.dram_tensor("ident", [128, 128], BF16, kind="ExternalInput").ap()
    x_rcvT = nc.dram_tensor("x_rcvT", [F, S], BF16, kind="ExternalInput").ap()
    y_perm = nc.dram_tensor("y_perm", [S, F], FP32, kind="ExternalOutput").ap()

    offs = np.zeros(rounds + 1, np.int64)
    np.cumsum(np.asarray(Khat), out=offs[1:])

    with TileContext(nc) as tc:
        import contextlib
        ctx = contextlib.ExitStack()
        with ctx:
            const = ctx.enter_context(tc.tile_pool(name="const", bufs=1))
            wv_s = const.tile([FW, J], BF16)
            nc.sync.dma_start(out=wv_s[:], in_=wv10[:])
            mc_s = const.tile([128, 2, F], BF16)
            nc.sync.dma_start(out=mc_s[:], in_=mcat[:].rearrange("p (i f) -> p i f", i=2))
            wm2_s = const.tile([F, F], BF16)
            nc.sync.dma_start(out=wm2_s[:], in_=wm2[:])
            id_s = const.tile([128, 128], BF16)
            nc.sync.dma_start(out=id_s[:], in_=ident[:])
            xrT_s = const.tile([F, S], BF16)

            NP = rounds // 2          # pair count; Khat[2p] == Khat[2p+1]

            with tc.tile_pool(name="ein", bufs=3) as ein, \
                 tc.tile_pool(name="mid", bufs=2) as mid, \
                 tc.tile_pool(name="payp", bufs=3) as payp, \
                 tc.tile_pool(name="gp", bufs=6) as gp, \
                 tc.tile_pool(name="outp", bufs=3) as outp, \
                 tc.tile_pool(name="psc", bufs=2, space="PSUM") as pscp, \
                 tc.tile_pool(name="pu", bufs=2, space="PSUM") as pup, \
                 tc.tile_pool(name="pt", bufs=2, space="PSUM") as ptp, \
                 tc.tile_pool(name="py", bufs=2, space="PSUM") as pyp:

                state = {}

                def dma_in(p):
                    K = Khat[2 * p]
                    o = int(offs[2 * p])
                    xeT_t = ein.tile([FW, 2 * K * 128], BF16, tag="xeT")
                    nc.sync.dma_start(out=xeT_t[:],
                                      in_=xeT[:, o * 128:(o + 2 * K) * 128])
                    xE_t = ein.tile([128, 2, K, F], BF16, tag="xE")
                    nc.sync.dma_start(
                        out=xE_t[:],
                        in_=xE[:, o * F:(o + 2 * K) * F].rearrange(
                            "p (c k f) -> p c k f", c=2, k=K))
                    state[('d', p)] = (xeT_t, xE_t)

                def stage_a1(p):
                    K = Khat[2 * p]
                    xeT_t, xE_t = state.pop(('d', p))
                    psc = pscp.tile([128, 2, K, J], FP32, space="PSUM",
                                    tag="psc")
                    for c in range(2):
                        for k in range(K):
                            nc.tensor.matmul(
                                out=psc[:, c, k, :],
                                lhsT=xeT_t[:, (c * K + k) * 128:
                                           (c * K + k + 1) * 128],
                                rhs=wv_s[:], start=True, stop=True,
                                skip_group_check=True)
                    w_t = mid.tile([128, 2, K, J], BF16, tag="w")
                    nc.scalar.activation(out=w_t[:], in_=psc[:], func=ACTF.Exp)
                    state[('a1', p)] = (w_t, xE_t)

                def stage_a2(p):
                    K = Khat[2 * p]
                    w_t, xE_t = state.pop(('a1', p))
                    d_t = mid.tile([128, 2, J], FP32, tag="d")
                    nc.vector.tensor_reduce(
                        out=d_t[:], in_=w_t[:].rearrange("p c k j -> p c j k"),
                        axis=AXIS.X, op=ALU.add)
                    rcp_t = mid.tile([128, 2, J], FP32, tag="rcp")
                    nc.vector.reciprocal(out=rcp_t[:], in_=d_t[:])
                    rcpb_t = mid.tile([128, 2, J], BF16, tag="rcpb")
                    nc.vector.tensor_copy(out=rcpb_t[:], in_=rcp_t[:])
                    gate_t = gp.tile([128, 2], FP32, tag="gate")
                    nc.vector.tensor_scalar(out=gate_t[:],
                                            in0=d_t[:, :, 0],
                                            scalar1=1e-8, scalar2=None,
                                            op0=ALU.is_gt)
                    al_t = mid.tile([128, 2, K, J], BF16, tag="al")
                    nc.vector.tensor_tensor(
                        out=al_t[:], in0=w_t[:],
                        in1=rcpb_t[:, :, None, :].to_broadcast([128, 2, K, J]),
                        op=ALU.mult)
                    be_t = mid.tile([128, 2, K, H], BF16, tag="be")
                    nc.vector.tensor_tensor(out=be_t[:],
                                            in0=al_t[:, :, :, 0:H],
                                            in1=al_t[:, :, :, H:2 * H],
                                            op=ALU.add)
                    br_t = mid.tile([128, 2, K, H, 16], BF16, tag="br")
                    nc.scalar.copy(
                        out=br_t[:],
                        in_=be_t[:, :, :, :, None].to_broadcast(
                            [128, 2, K, H, 16]))
                    pay_t = payp.tile([128, 2, K, H, F], BF16, tag="pay")
                    for fo in range(F // 16):
                        nc.vector.tensor_tensor(
                            out=pay_t[:, :, :, :, fo * 16:(fo + 1) * 16],
                            in0=br_t[:],
                            in1=xE_t[:, :, :, None, fo * 16:(fo + 1) * 16]
                                .to_broadcast([128, 2, K, H, 16]),
                            op=ALU.mult)
                    state[p] = (pay_t, gate_t)

                def stage_b1(p):
                    K = Khat[2 * p]
                    pay_t, gate_t = state.pop(p)
                    pu = pup.tile([128, 2, H * F], FP32, space="PSUM",
                                  tag="pu")
                    for c in range(2):
                        for k in range(K):
                            nc.tensor.matmul(
                                out=pu[:, c, :],
                                lhsT=id_s[:],
                                rhs=pay_t[:, c, k].rearrange(
                                    "p h f -> p (h f)"),
                                start=(k == 0), stop=(k == K - 1),
                                skip_group_check=(c == 1))
                    usb = outp.tile([128, 2, H * F], BF16, tag="usb")
                    nc.scalar.copy(out=usb[:], in_=pu[:])
                    state[('b1', p)] = (usb, gate_t)

                def stage_b2(p):
                    usb, gate_t = state.pop(('b1', p))
                    pt = ptp.tile([128, 4, 128], BF16, space="PSUM", tag="pt")
                    for i in range(4):
                        nc.tensor.transpose(
                            out=pt[:, i, :],
                            in_=usb[:, i // 2, (i % 2) * 128:
                                    (i % 2 + 1) * 128],
                            identity=id_s[:])
                    utsb = outp.tile([128, 4, 128], BF16, tag="utsb")
                    nc.scalar.copy(out=utsb[:], in_=pt[:])
                    state[('b2', p)] = (utsb, gate_t)

                def stage_b3(p):
                    utsb, gate_t = state.pop(('b2', p))
                    py = pyp.tile([128, 2, 2, F], FP32, space="PSUM", tag="py")
                    for c in range(2):
                        r = 2 * p + c
                        nc.tensor.matmul(out=py[:, c, 0, :],
                                         lhsT=utsb[:, 2 * c, :],
                                         rhs=mc_s[:, 0, :],
                                         start=True, stop=False,
                                         skip_group_check=(c == 1))
                        nc.tensor.matmul(out=py[:, c, 0, :],
                                         lhsT=utsb[:, 2 * c + 1, :],
                                         rhs=mc_s[:, 1, :],
                                         start=False, stop=False)
                        # residual x via I64 rhs, accumulated into the bank
                        nc.tensor.matmul(out=py[:, c, 0, :],
                                         lhsT=xrT_s[:, r * 128:(r + 1) * 128],
                                         rhs=id_s[0:F, 0:F],
                                         start=False, stop=True)
                        nc.tensor.matmul(out=py[:, c, 1, :],
                                         lhsT=xrT_s[:, r * 128:(r + 1) * 128],
                                         rhs=wm2_s[:], start=True, stop=True,
                                         skip_group_check=True)
                    accp = outp.tile([128, 2, F], BF16, tag="accp")
                    for c in range(2):
                        nc.scalar.mul(accp[:, c, :], py[:, c, 1, :],
                                      gate_t[:, c:c + 1])
                    yf = outp.tile([128, 2, F], FP32, tag="yf")
                    nc.vector.tensor_tensor(out=yf[:], in0=py[:, :, 0, :],
                                            in1=accp[:], op=ALU.add)
                    nc.sync.dma_start(
                        out=y_perm[2 * p * 128:(2 * p + 2) * 128, :]
                            .rearrange("(c s) f -> s c f", c=2),
                        in_=yf[:])

                for p in range(min(3, NP)):
                    dma_in(p)
                # xrT preload deferred so it doesn't block warmup chunk DMAs
                nc.sync.dma_start(out=xrT_s[:], in_=x_rcvT[:])
                for p in range(min(2, NP)):
                    stage_a1(p)
                    stage_a2(p)
                stage_b1(0)
                # skewed pipeline: every PE op consumes only >=1-iteration-old
                # producers, so the PE never waits mid-iteration.
                for it in range(NP + 1):
                    if it + 3 < NP:
                        dma_in(it + 3)
                    if it + 2 < NP:
                        stage_a1(it + 2)
                        stage_a2(it + 2)
                    if it < NP:
                        stage_b2(it)
                    if it - 1 >= 0:
                        stage_b3(it - 1)
                    if it + 1 < NP:
                        stage_b1(it + 1)
    split_sync_waits(nc, maxw=1)
    nc.finalize()
    return nc


def make_in_maps(meta, w_proj, rs, ts, rds, w_out, F=64, H=4):
    FW = F + 2
    wv10 = np.zeros((FW, 2 * H), np.float32)
    for h in range(H):
        wv10[0:F, h] = w_proj[h] @ ts[h]
        wv10[0:F, H + h] = w_proj[h] @ rs[h]
    wv10[F, H:2 * H] = -float(rds)
    wv10[F + 1, :] = -30.0
    mcat = np.zeros((128, 2 * F), np.float32)
    for h in range(H):
        Mh = 0.25 * (w_proj[h] @ w_out)
        i, jo = divmod(h, 2)
        mcat[jo * F:(jo + 1) * F, i * F:(i + 1) * F] = Mh
    wm2 = (-0.5 * w_proj.sum(axis=0) @ w_out).astype(np.float32)
    ident = np.eye(128, dtype=np.float32)
    in_maps = []
    for k in range(NC_CORES):
        c = meta['per_core'][k]
        in_maps.append({
            "xeT": c['xeT'], "xE": c['xE'],
            "wv10": wv10.astype(bf16), "mcat": mcat.astype(bf16),
            "wm2": wm2.astype(bf16), "ident": ident.astype(bf16),
            "x_rcvT": c['x_rcvT'],
        })
    return in_maps


def assemble(meta, results):
    N, F = meta['N'], meta['F']
    y = np.zeros((N, F), np.float32)
    for k in range(NC_CORES):
        c = meta['per_core'][k]
        yp = results[k]['y_perm']
        for r in range(meta['rounds']):
            n = c['recvs'][r]
            has = n >= 0
            y[n[has]] = yp[r * 128:(r + 1) * 128][has]
    return y


install()

_CACHE = {}
_LAST = {}


def kernel(x, edge_index, edge_vec, edge_len, w_proj, radial_score,
           tangential_score, radial_distance_scale, w_out):
    x = np.asarray(x, np.float32)
    edge_index = np.asarray(edge_index)
    edge_len = np.asarray(edge_len, np.float32)
    w_proj = np.asarray(w_proj, np.float32)
    rs = np.asarray(radial_score, np.float32)
    ts = np.asarray(tangential_score, np.float32)
    rds = np.float32(np.asarray(radial_distance_scale))
    w_out_ = np.asarray(w_out, np.float32)

    F = x.shape[1]
    H = w_proj.shape[0]
    meta = host_prep(x, edge_index, edge_len, F=F)
    key = (meta['Khat'], meta['rounds'], F, H)
    if key not in _CACHE:
        _CACHE[key] = build_program(meta['Khat'], meta['rounds'],
                                    meta['T_tot'], F=F, H=H)
    nc = _CACHE[key]
    in_maps = make_in_maps(meta, w_proj, rs, ts, rds, w_out_, F=F, H=H)
    from concourse.bass_utils import run_bass_kernel_spmd
    res = run_bass_kernel_spmd(nc, in_maps, list(range(NC_CORES)))
    _LAST['nc'] = nc
    _LAST['in_maps'] = in_maps
    _LAST['meta'] = meta
    y = assemble(meta, [res.results[i] for i in range(NC_CORES)])
    return y.astype(np.float32)


# revision 8
# speedup vs baseline: 1.0433x; 1.0433x over previous
"""Trainium2 Bass kernel v3 for nn_DenseFlashAttention_16123307229343
(GNN segment-softmax message passing). kernel(**inputs) -> np.ndarray.

v3 redesign vs v2 (one-hot scatter baseline):
- ELL layout: receivers globally degree-sorted into chunks of 128; tile k
  of a chunk holds the k-th edge of each of the chunk's 128 receivers, so
  receiver == partition slot. One-hot scatter matrices are gone entirely
  (16.4 MB DMA/core + 2 of 3 matmuls/tile eliminated); segment sums become
  free-dim reductions / identity-matmul PSUM accumulation.
- Normalize-before-scatter: alpha = w/d per edge via per-partition
  broadcast (d is per-slot), which merges tangential+radial per head into
  beta (4 blocks instead of 8) and shrinks the scatter payload 2x.
- Raw-x payload: the per-edge 264-wide projection matmul is replaced by an
  8-col score matmul; the per-head projection (folded with w_out) is
  applied per chunk (128 receivers) after the reduction, via 2 stacked
  128-contraction matmuls on transposed U.
- Chunk K-profile is shared across all 8 cores (round-robin deal by degree
  rank, padded to the round max), so one SPMD program serves all cores
  with ~0.5% padding.
"""
import sys, math
sys.path.insert(0, '/opt/trn_rl_repo')

import numpy as np
import ml_dtypes

import concourse.mybir as mybir
import concourse.bass as bass
from concourse.tile import TileContext
from concourse.vector_clock import ScopedClock

bf16 = ml_dtypes.bfloat16
FP32 = mybir.dt.float32
BF16 = mybir.dt.bfloat16
ALU = mybir.AluOpType
ACTF = mybir.ActivationFunctionType
AXIS = mybir.AxisListType

MAXW = 1


def _patched_drain_and_barrier(self, tick_clock, wait_clock):
    nc = self.nc
    drain_inst = nc.sync.drain()
    wait_clock.add_sem_waits(drain_inst.ins, ScopedClock({None: tick_clock.global_clock}))
    si = drain_inst.ins.sync_info
    waits = list(si.on_wait) if si is not None else []
    if len(waits) > MAXW:
        si.on_wait = waits[:MAXW]
        rest = waits[MAXW:]
        for i in range(0, len(rest), MAXW):
            d2 = nc.sync.drain()
            d2.ins.sync_info = mybir.SyncInfo(on_wait=rest[i:i+MAXW], on_update=[])
    nc.all_engine_barrier()
    popped = nc._tile_sem_poison_stack.pop()
    assert popped is self._sem_poison
    nc.clear_and_free_semaphores(list(self.sems.allocated().values()))
    nc.all_engine_barrier()


def install():
    TileContext._drain_and_barrier = _patched_drain_and_barrier


_ctr = [0]


def split_sync_waits(nc, maxw=1):
    """The walrus build in this container rejects instructions carrying more
    than one sync wait. Hoist extra waits onto carriers inserted immediately
    before the instruction on the same engine."""
    for f in nc.m.functions:
        for blk in f.blocks:
            lst = blk.instructions
            i = 0
            while i < len(lst):
                ins = lst[i]
                si = ins.sync_info
                if si is None:
                    i += 1
                    continue
                waits = list(si.on_wait)
                if len(waits) <= maxw:
                    i += 1
                    continue
                si.on_wait = waits[-maxw:]
                rest = waits[:-maxw]
                carriers = []
                for j in range(0, len(rest), maxw):
                    _ctr[0] += 1
                    nop = mybir.InstEventSemaphore(name=f"waitnop_{_ctr[0]}", ins=[], outs=[])
                    nop.engine = ins.engine
                    nop.sync_info = mybir.SyncInfo(on_wait=rest[j:j + maxw],
                                                   on_update=[])
                    nc.register_instruction(nop, overwrite=True)
                    carriers.append(nop)
                for k, nop in enumerate(carriers):
                    lst.insert(i + k, nop)
                i += len(carriers) + 1


NC_CORES = 8


def host_prep(x, edge_index, edge_len, F=64):
    """ELL layout with a core-shared K profile. Returns per-core input
    tensors plus the metadata needed to assemble the output."""
    N = x.shape[0]
    E = edge_index.shape[1]
    snd = edge_index[0].astype(np.int64)
    rcv = edge_index[1].astype(np.int64)
    deg = np.bincount(rcv, minlength=N)
    eorder = np.argsort(rcv, kind='stable')
    starts = np.zeros(N + 1, np.int64)
    np.cumsum(deg, out=starts[1:])

    rorder = np.argsort(-deg, kind='stable')
    nch_total = (N + 127) // 128
    # pad chunk count so the per-core round count is even (pair processing)
    nch_pad = ((nch_total + 2 * NC_CORES - 1) // (2 * NC_CORES)) * 2 * NC_CORES
    rounds = nch_pad // NC_CORES
    recv_of_chunk = np.full((nch_pad, 128), -1, np.int64)
    recv_of_chunk.reshape(-1)[:N] = rorder
    # chunk max degree; profile = per-round max so all cores share it
    degc = np.where(recv_of_chunk >= 0, deg[np.clip(recv_of_chunk, 0, N - 1)], 0)
    Kc = np.maximum(degc.max(axis=1), 1)
    Khat = np.array([int(Kc[r * NC_CORES:(r + 1) * NC_CORES].max())
                     for r in range(rounds)], np.int64)
    # ascending-K round order: pipeline warms up and drains on small chunks
    rord = np.argsort(Khat, kind='stable')
    Khat = Khat[rord]
    Kreal = Khat.copy()              # per-round true K (pre pair padding)
    # pair adjacent rounds; both chunks of a pair padded to the pair max
    Kpair = np.maximum(Khat[0::2], Khat[1::2])
    Khat = np.repeat(Kpair, 2)
    T_tot = int(Khat.sum())

    xb = np.zeros((N + 1, F), dtype=bf16)
    xb[:N] = x.astype(bf16)
    lenb = edge_len.astype(np.float32)

    offs = np.zeros(rounds + 1, np.int64)
    np.cumsum(Khat, out=offs[1:])

    per_core = []
    for core in range(NC_CORES):
        xeT = np.zeros((F + 2, T_tot * 128), dtype=bf16)
        xE = np.zeros((128, T_tot * F), dtype=bf16)
        xr = np.zeros((rounds * 128, F), np.float32)
        recvs = np.empty((rounds, 128), np.int64)
        for r in range(rounds):
            c = core + int(rord[r]) * NC_CORES
            K = int(Khat[r])
            n = recv_of_chunk[c]                       # [128]
            recvs[r] = n
            nc_ = np.clip(n, 0, N - 1)
            d = np.where(n >= 0, deg[nc_], 0)          # [128]
            ks = np.arange(K)
            valid = ks[None, :] < d[:, None]           # [128, K]
            eidx = starts[nc_][:, None] + np.minimum(ks[None, :], np.maximum(d[:, None] - 1, 0))
            e = eorder[np.clip(eidx, 0, E - 1)]
            sg = np.where(valid, snd[e], N)            # pad -> row N (zeros)
            blk = xb[sg]                               # [128, K, F] bf16
            lg = np.where(valid, lenb[e], 0.0).astype(bf16)
            mk = (~valid).astype(bf16)
            o = offs[r]
            xE[:, o * F:(o + K) * F] = blk.reshape(128, K * F)
            xeT[0:F, o * 128:(o + K) * 128] = blk.transpose(2, 1, 0).reshape(F, K * 128)
            xeT[F, o * 128:(o + K) * 128] = lg.T.reshape(K * 128)
            xeT[F + 1, o * 128:(o + K) * 128] = mk.T.reshape(K * 128)
            has = n >= 0
            xr[r * 128:(r + 1) * 128][has] = x[n[has]]
        xrT = np.ascontiguousarray(xr.T).astype(bf16)
        per_core.append(dict(xeT=xeT, xE=xE, x_rcvT=xrT, recvs=recvs))
    return dict(Khat=tuple(int(k) for k in Khat),
                Kreal=tuple(int(k) for k in Kreal), rounds=rounds,
                T_tot=T_tot, offs=offs, per_core=per_core, N=N, F=F)


def build_program(Khat, Kreal, rounds, T_tot, F=64, H=4):
    nc = bass.Bass("TRN2", target_bir_lowering=False, debug=False,
                   num_devices=NC_CORES)
    FW = F + 2                       # 66 contraction rows (x, len, mask)
    J = 2 * H                        # 8 score columns
    S = rounds * 128                 # receiver slots per core

    xeT = nc.dram_tensor("xeT", [FW, T_tot * 128], BF16, kind="ExternalInput").ap()
    xE = nc.dram_tensor("xE", [128, T_tot * F], BF16, kind="ExternalInput").ap()
    wv10 = nc.dram_tensor("wv10", [FW, J], BF16, kind="ExternalInput").ap()
    mcat = nc.dram_tensor("mcat", [128, 2 * F], BF16, kind="ExternalInput").ap()
    wm2 = nc.dram_tensor("wm2", [F, F], BF16, kind="ExternalInput").ap()
    ident = nc.dram_tensor("ident", [128, 128], BF16, kind="ExternalInput").ap()
    x_rcvT = nc.dram_tensor("x_rcvT", [F, S], BF16, kind="ExternalInput").ap()
    y_perm = nc.dram_tensor("y_perm", [S, F], FP32, kind="ExternalOutput").ap()

    offs = np.zeros(rounds + 1, np.int64)
    np.cumsum(np.asarray(Khat), out=offs[1:])

    with TileContext(nc) as tc:
        import contextlib
        ctx = contextlib.ExitStack()
        with ctx:
            const = ctx.enter_context(tc.tile_pool(name="const", bufs=1))
            wv_s = const.tile([FW, J], BF16)
            nc.sync.dma_start(out=wv_s[:], in_=wv10[:])
            mc_s = const.tile([128, 2, F], BF16)
            nc.sync.dma_start(out=mc_s[:], in_=mcat[:].rearrange("p (i f) -> p i f", i=2))
            wm2_s = const.tile([F, F], BF16)
            nc.sync.dma_start(out=wm2_s[:], in_=wm2[:])
            id_s = const.tile([128, 128], BF16)
            nc.sync.dma_start(out=id_s[:], in_=ident[:])
            xrT_s = const.tile([F, S], BF16)

            NP = rounds // 2          # pair count; Khat[2p] == Khat[2p+1]

            with tc.tile_pool(name="ein", bufs=3) as ein, \
                 tc.tile_pool(name="mid", bufs=2) as mid, \
                 tc.tile_pool(name="payp", bufs=3) as payp, \
                 tc.tile_pool(name="gp", bufs=8) as gp, \
                 tc.tile_pool(name="outp", bufs=3) as outp, \
                 tc.tile_pool(name="psc", bufs=2, space="PSUM") as pscp, \
                 tc.tile_pool(name="pu", bufs=2, space="PSUM") as pup, \
                 tc.tile_pool(name="pt", bufs=2, space="PSUM") as ptp, \
                 tc.tile_pool(name="py", bufs=2, space="PSUM") as pyp:

                state = {}

                def dma_in(p):
                    K = Khat[2 * p]
                    o = int(offs[2 * p])
                    xeT_t = ein.tile([FW, 2 * K * 128], BF16, tag="xeT")
                    nc.sync.dma_start(out=xeT_t[:],
                                      in_=xeT[:, o * 128:(o + 2 * K) * 128])
                    xE_t = ein.tile([128, 2, K, F], BF16, tag="xE")
                    nc.sync.dma_start(
                        out=xE_t[:],
                        in_=xE[:, o * F:(o + 2 * K) * F].rearrange(
                            "p (c k f) -> p c k f", c=2, k=K))
                    state[('d', p)] = (xeT_t, xE_t)

                def stage_a1(p):
                    K = Khat[2 * p]
                    xeT_t, xE_t = state.pop(('d', p))
                    psc = pscp.tile([128, 2, K, J], FP32, space="PSUM",
                                    tag="psc")
                    for c in range(2):
                        for k in range(K):
                            nc.tensor.matmul(
                                out=psc[:, c, k, :],
                                lhsT=xeT_t[:, (c * K + k) * 128:
                                           (c * K + k + 1) * 128],
                                rhs=wv_s[:], start=True, stop=True,
                                skip_group_check=True)
                    w_t = mid.tile([128, 2, K, J], BF16, tag="w")
                    nc.scalar.activation(out=w_t[:], in_=psc[:], func=ACTF.Exp)
                    state[('a1', p)] = (w_t, xE_t)

                def stage_a2(p):
                    K = Khat[2 * p]
                    w_t, xE_t = state.pop(('a1', p))
                    d_t = mid.tile([128, 2, J], FP32, tag="d")
                    nc.vector.tensor_reduce(
                        out=d_t[:], in_=w_t[:].rearrange("p c k j -> p c j k"),
                        axis=AXIS.X, op=ALU.add)
                    rcp_t = mid.tile([128, 2, J], FP32, tag="rcp")
                    nc.vector.reciprocal(out=rcp_t[:], in_=d_t[:])
                    rcpb_t = mid.tile([128, 2, J], BF16, tag="rcpb")
                    nc.vector.tensor_copy(out=rcpb_t[:], in_=rcp_t[:])
                    gate_t = gp.tile([128, 2], FP32, tag="gate")
                    nc.vector.tensor_scalar(out=gate_t[:],
                                            in0=d_t[:, :, 0],
                                            scalar1=1e-8, scalar2=None,
                                            op0=ALU.is_gt)
                    al_t = mid.tile([128, 2, K, J], BF16, tag="al")
                    nc.vector.tensor_tensor(
                        out=al_t[:], in0=w_t[:],
                        in1=rcpb_t[:, :, None, :].to_broadcast([128, 2, K, J]),
                        op=ALU.mult)
                    be_t = mid.tile([128, 2, K, H], BF16, tag="be")
                    nc.vector.tensor_tensor(out=be_t[:],
                                            in0=al_t[:, :, :, 0:H],
                                            in1=al_t[:, :, :, H:2 * H],
                                            op=ALU.add)
                    br_t = mid.tile([128, 2, K, H, 16], BF16, tag="br")
                    nc.scalar.copy(
                        out=br_t[:],
                        in_=be_t[:, :, :, :, None].to_broadcast(
                            [128, 2, K, H, 16]))
                    pay_t = payp.tile([128, 2, K, H, F], BF16, tag="pay")
                    for fo in range(F // 16):
                        nc.vector.tensor_tensor(
                            out=pay_t[:, :, :, :, fo * 16:(fo + 1) * 16],
                            in0=br_t[:],
                            in1=xE_t[:, :, :, None, fo * 16:(fo + 1) * 16]
                                .to_broadcast([128, 2, K, H, 16]),
                            op=ALU.mult)
                    state[p] = (pay_t, gate_t)

                def stage_b1(p):
                    pay_t, gate_t = state.pop(p)
                    pu = pup.tile([128, 2, H * F], FP32, space="PSUM",
                                  tag="pu")
                    for c in range(2):
                        KR = Kreal[2 * p + c]
                        for k in range(KR):
                            # pad tiles beyond the chunk's real K have an
                            # exactly-zero payload (x=0) and are skipped
                            nc.tensor.matmul(
                                out=pu[:, c, :],
                                lhsT=id_s[:],
                                rhs=pay_t[:, c, k].rearrange(
                                    "p h f -> p (h f)"),
                                start=(k == 0), stop=(k == KR - 1),
                                skip_group_check=(c == 1))
                    state[('b1', p)] = (pu, gate_t)

                def stage_b1b(p):
                    pu, gate_t = state.pop(('b1', p))
                    usb = outp.tile([128, 2, H * F], BF16, tag="usb")
                    nc.scalar.copy(out=usb[:], in_=pu[:])
                    state[('usb', p)] = (usb, gate_t)

                def stage_b2(p):
                    usb, gate_t = state.pop(('usb', p))
                    pt = ptp.tile([128, 4, 128], BF16, space="PSUM", tag="pt")
                    for i in range(4):
                        nc.tensor.transpose(
                            out=pt[:, i, :],
                            in_=usb[:, i // 2, (i % 2) * 128:
                                    (i % 2 + 1) * 128],
                            identity=id_s[:])
                    state[('b2', p)] = (pt, gate_t)

                def stage_b2b(p):
                    pt, gate_t = state.pop(('b2', p))
                    utsb = outp.tile([128, 4, 128], BF16, tag="utsb")
                    nc.scalar.copy(out=utsb[:], in_=pt[:])
                    state[('utsb', p)] = (utsb, gate_t)

                def stage_b3a(p):
                    utsb, gate_t = state.pop(('utsb', p))
                    py = pyp.tile([128, 2, 2, F], FP32, space="PSUM", tag="py")
                    for c in range(2):
                        r = 2 * p + c
                        nc.tensor.matmul(out=py[:, c, 0, :],
                                         lhsT=utsb[:, 2 * c, :],
                                         rhs=mc_s[:, 0, :],
                                         start=True, stop=False,
                                         skip_group_check=(c == 1))
                        nc.tensor.matmul(out=py[:, c, 0, :],
                                         lhsT=utsb[:, 2 * c + 1, :],
                                         rhs=mc_s[:, 1, :],
                                         start=False, stop=False)
                        # residual x via I64 rhs, accumulated into the bank
                        nc.tensor.matmul(out=py[:, c, 0, :],
                                         lhsT=xrT_s[:, r * 128:(r + 1) * 128],
                                         rhs=id_s[0:F, 0:F],
                                         start=False, stop=True)
                        nc.tensor.matmul(out=py[:, c, 1, :],
                                         lhsT=xrT_s[:, r * 128:(r + 1) * 128],
                                         rhs=wm2_s[:], start=True, stop=True,
                                         skip_group_check=True)
                    state[('py', p)] = (py, gate_t)

                def stage_b3b(p):
                    py, gate_t = state.pop(('py', p))
                    accp = outp.tile([128, 2, F], BF16, tag="accp")
                    for c in range(2):
                        nc.scalar.mul(accp[:, c, :], py[:, c, 1, :],
                                      gate_t[:, c:c + 1])
                    yf = outp.tile([128, 2, F], FP32, tag="yf")
                    nc.vector.tensor_tensor(out=yf[:], in0=py[:, :, 0, :],
                                            in1=accp[:], op=ALU.add)
                    nc.sync.dma_start(
                        out=y_perm[2 * p * 128:(2 * p + 2) * 128, :]
                            .rearrange("(c s) f -> s c f", c=2),
                        in_=yf[:])

                for p in range(min(3, NP)):
                    dma_in(p)
                # xrT preload deferred so it doesn't block warmup chunk DMAs
                nc.sync.dma_start(out=xrT_s[:], in_=x_rcvT[:])
                for p in range(min(2, NP)):
                    stage_a1(p)
                    stage_a2(p)
                stage_b1(0)
                # fully skewed pipeline: pair p runs a@p-2, idU@p-1, usb@p,
                # T@p+1, utsb@p+2, y@p+3, accp/yf/out@p+4 -- every
                # instruction's producers are >=1 full iteration old, so no
                # engine ever waits mid-iteration.
                for it in range(NP + 5):
                    if it + 3 < NP:
                        dma_in(it + 3)
                    if it + 2 < NP:
                        stage_a1(it + 2)
                        stage_a2(it + 2)
                    if 0 <= it + 1 < NP and it + 1 > 0:
                        stage_b1(it + 1)
                    if 0 <= it < NP:
                        stage_b1b(it)
                    if 0 <= it - 1 < NP:
                        stage_b2(it - 1)
                    if 0 <= it - 2 < NP:
                        stage_b2b(it - 2)
                    if 0 <= it - 3 < NP:
                        stage_b3a(it - 3)
                    if 0 <= it - 4 < NP:
                        stage_b3b(it - 4)
    split_sync_waits(nc, maxw=1)
    nc.finalize()
    return nc


def make_in_maps(meta, w_proj, rs, ts, rds, w_out, F=64, H=4):
    FW = F + 2
    wv10 = np.zeros((FW, 2 * H), np.float32)
    for h in range(H):
        wv10[0:F, h] = w_proj[h] @ ts[h]
        wv10[0:F, H + h] = w_proj[h] @ rs[h]
    wv10[F, H:2 * H] = -float(rds)
    wv10[F + 1, :] = -30.0
    mcat = np.zeros((128, 2 * F), np.float32)
    for h in range(H):
        Mh = 0.25 * (w_proj[h] @ w_out)
        i, jo = divmod(h, 2)
        mcat[jo * F:(jo + 1) * F, i * F:(i + 1) * F] = Mh
    wm2 = (-0.5 * w_proj.sum(axis=0) @ w_out).astype(np.float32)
    ident = np.eye(128, dtype=np.float32)
    in_maps = []
    for k in range(NC_CORES):
        c = meta['per_core'][k]
        in_maps.append({
            "xeT": c['xeT'], "xE": c['xE'],
            "wv10": wv10.astype(bf16), "mcat": mcat.astype(bf16),
            "wm2": wm2.astype(bf16), "ident": ident.astype(bf16),
            "x_rcvT": c['x_rcvT'],
        })
    return in_maps


def assemble(meta, results):
    N, F = meta['N'], meta['F']
    y = np.zeros((N, F), np.float32)
    for k in range(NC_CORES):
        c = meta['per_core'][k]
        yp = results[k]['y_perm']
        for r in range(meta['rounds']):
            n = c['recvs'][r]
            has = n >= 0
            y[n[has]] = yp[r * 128:(r + 1) * 128][has]
    return y


install()

_CACHE = {}
_LAST = {}


def kernel(x, edge_index, edge_vec, edge_len, w_proj, radial_score,
           tangential_score, radial_distance_scale, w_out):
    x = np.asarray(x, np.float32)
    edge_index = np.asarray(edge_index)
    edge_len = np.asarray(edge_len, np.float32)
    w_proj = np.asarray(w_proj, np.float32)
    rs = np.asarray(radial_score, np.float32)
    ts = np.asarray(tangential_score, np.float32)
    rds = np.float32(np.asarray(radial_distance_scale))
    w_out_ = np.asarray(w_out, np.float32)

    F = x.shape[1]
    H = w_proj.shape[0]
    meta = host_prep(x, edge_index, edge_len, F=F)
    key = (meta['Khat'], meta['Kreal'], meta['rounds'], F, H)
    if key not in _CACHE:
        _CACHE[key] = build_program(meta['Khat'], meta['Kreal'],
                                    meta['rounds'], meta['T_tot'], F=F, H=H)
    nc = _CACHE[key]
    in_maps = make_in_maps(meta, w_proj, rs, ts, rds, w_out_, F=F, H=H)
    from concourse.bass_utils import run_bass_kernel_spmd
    res = run_bass_kernel_spmd(nc, in_maps, list(range(NC_CORES)))
    _LAST['nc'] = nc
    _LAST['in_maps'] = in_maps
    _LAST['meta'] = meta
    y = assemble(meta, [res.results[i] for i in range(NC_CORES)])
    return y.astype(np.float32)
